# revision 40
# baseline (speedup 1.0000x reference)
"""Trainium2 Bass kernel for the capsule-routing layer (nn_CapsConvLayer).

Math (reference):
  u_j_i[b,i,c,o] = sum_k W[i,c,o,k] * x[b,k,i]
  b_ic = 0
  3x:  c = softmax(b, axis=i)
       s[b,c,o]  = sum_i c[i,c] * u_j_i[b,i,c,o]
       out       = squash_over_c(s)
       agr[i,c]  = sum_{b,o} u_j_i[b,i,c,o] * out[b,c,o] / B
       b += agr
  return out[..., None]

Strategy: shard the input-capsule dim I across 8 cores (I_loc=144).  u_j_i is
never materialized; s is computed as one fused matmul with contraction over
(i,k) per core:  s~[b,co] = sum_ik u1[ik,b] * (exp(b)[i,c] * W1[ik,co]).

Cross-core reduction of the s~ partials (iterations 1 and 2) is done with
remote SBUF->SBUF DMA instead of ncfw collective_compute: ONE
remote_dma_broadcast per exchange with all 8 destinations real (dummy slots
cost as much SDMA descriptor time as real ones), writing slot <own core id>
on every receiver via a register-offset out_ap.  Each core then sums the 8
slots locally on DVE.  This avoids the ncfw RDH latency (~13us) and the HBM
bounce entirely (~6.5us per 660KB exchange, descriptor-rate bound).

The one-time collectives init barrier is still needed ONCE as an *init
fence*: a tiny AllToAll fired as the first instruction.  The first remote
trigger is gated on its completion so no remote semaphore increment can
arrive at a peer before that peer's NRT-preamble sema_reset has run
(late-start cores would otherwise lose increments -> deadlock).  The fence
runs on the CC stream concurrently with the input loads + it1 matmuls.

agr is computed locally per core via T[co,ik] = sum_b out[b,co]*u2[b,ik]
(PE), M = W2 .* T (DVE), then two small PE contractions (class-selector and
k-sum-replicate).  Routing state is kept multiplicatively:
cW_new = cW_old .* exp(agr_repl).  The final iteration only needs partial
s~3 + Z3, which the host sums and squashes (tiny [256,10,16]).
"""

import os
import sys

sys.path.insert(0, "/opt/trn_rl_repo")

import ml_dtypes
import numpy as np

BF = ml_dtypes.bfloat16

import concourse.bacc as bacc
import concourse.bass as bass
import concourse.mybir as mybir
import concourse.tile as tile
from concourse import bass_utils

# Problem constants (hardcoded per contract)
B, K, I, C, O = 256, 8, 1152, 10, 16
NCORES = 8
ILOC = I // NCORES          # 144
IK = ILOC * K               # 1152 contraction size per core
NT = IK // 128              # 9 partition tiles
CO = C * O                  # 160
NB = B // 128               # 2 batch chunks
F1 = NB * CO                # 320  xch1 payload cols
F2 = NB * CO + 16           # 336  xch2 payload cols (z in col 320)
F32 = mybir.dt.float32
BF16 = mybir.dt.bfloat16
ADD = mybir.AluOpType.add
MULT = mybir.AluOpType.mult

SIM_SAFE = bool(int(os.environ.get("KERNEL_SIM_SAFE", "0")))

_CACHE = {}


def _build():
    nc = bacc.Bacc("TRN2", target_bir_lowering=False, debug=False,
                   enable_asserts=False, num_devices=NCORES,
                   num_swdge_queues=3)

    u1d = nc.dram_tensor("u1", [IK, B], BF16, kind="ExternalInput")
    w1d = nc.dram_tensor("w1", [IK, CO], BF16, kind="ExternalInput")
    rmatd = nc.dram_tensor("rmat", [128, 128], BF16, kind="ExternalInput")
    seld = nc.dram_tensor("sel", [CO, C], BF16, kind="ExternalInput")
    identd = nc.dram_tensor("ident", [128, 128], BF16, kind="ExternalInput")
    s3d = nc.dram_tensor("s3out", [B, CO], BF16, kind="ExternalOutput")
    z3d = nc.dram_tensor("z3out", [C, 1], F32, kind="ExternalOutput")
    DEBUG_GBUF = bool(int(os.environ.get("KERNEL_DEBUG_GBUF", "0")))
    gdbg = (nc.dram_tensor("gdbg", [128, 8 * F1], BF16, kind="ExternalOutput")
            if DEBUG_GBUF else None)
    sdbg = (nc.dram_tensor("sdbg", [128, F1], BF16, kind="ExternalOutput")
            if DEBUG_GBUF else None)
    s2dbg = (nc.dram_tensor("s2dbg", [128, F2], BF16, kind="ExternalOutput")
             if DEBUG_GBUF else None)
    odbg = (nc.dram_tensor("odbg", [128, NB * CO], BF16, kind="ExternalOutput")
            if DEBUG_GBUF else None)
    qdbg = (nc.dram_tensor("qdbg", [128, C], F32, kind="ExternalOutput")
            if DEBUG_GBUF else None)
    gdbg2 = (nc.dram_tensor("gdbg2", [128, 8 * F2], BF16, kind="ExternalOutput")
             if DEBUG_GBUF else None)

    with tile.TileContext(nc) as tc:
        with (
            tc.tile_pool(name="sb", bufs=1) as sb,
            tc.tile_pool(name="ps", bufs=2, space="PSUM") as ps,
            tc.tile_pool(name="ps2", bufs=2, space="PSUM") as ps2,
            tc.tile_pool(name="dram", bufs=1, space="DRAM") as dram,
        ):
            # persistent SBUF state
            u1sb = sb.tile([128, NT * B], BF16, tag="u1sb")     # [p, t*256+b]
            u2sb = sb.tile([128, NB * IK], BF16, tag="u2sb")    # [p, nb*1152+ik]
            w1sb = sb.tile([128, NT * CO], BF16, tag="w1sb")    # [p, t*160+co]
            w2sb = [sb.tile([80, IK], BF16, name=f"w2sb{g}", tag=f"w2sb{g}") for g in range(2)]
            cwsb = sb.tile([128, NT * CO], BF16, tag="cwsb")
            cexp = sb.tile([128, NT * C], BF16, tag="cexp")
            efac = sb.tile([128, NT * C], BF16, tag="efac")
            onesb = sb.tile([128, 1], BF16, tag="onesb")
            ones10 = sb.tile([16, 128], BF16, tag="ones10")
            rsb = sb.tile([128, 128], BF16, tag="rsb")
            selsb = [sb.tile([80, C], BF16, name=f"selsb{g}", tag=f"selsb{g}") for g in range(2)]
            idsb = sb.tile([128, 128], BF16, tag="idsb")
            spre1 = sb.tile([128, F1], BF16, tag="spre1")       # xch1 payload
            spre2 = sb.tile([128, F2], BF16, tag="spre2")       # xch2 payload
            gbuf1 = sb.tile([128, 8 * F1], BF16, tag="gbuf1")   # xch1 gather
            gbuf2 = sb.tile([128, 8 * F2], BF16, tag="gbuf2")   # xch2 gather
            ssb = sb.tile([128, F1], BF16, tag="ssb")           # s~ glob it1
            ssb2 = sb.tile([128, F2], BF16, tag="ssb2")         # s~ glob + z it2
            zdsb = sb.tile([16, 16], BF16, tag="zdsb")          # diag(z)
            qsb = sb.tile([128, C], F32, tag="qsb")
            outc = sb.tile([128, NB * CO], BF16, tag="outc")    # squashed caps
            agrk_sb = sb.tile([128, NT * C], BF16, tag="agrk_sb")
            msb = [sb.tile([80, IK], BF16, name=f"msb{g}", tag=f"msb{g}") for g in range(2)]
            sq = sb.tile([128, NB * CO], BF16, tag="sq")
            n2 = sb.tile([128, NB * O], F32, tag="n2")
            phi = sb.tile([128, NB * O], F32, tag="phi")
            actscr = sb.tile([1, 1], F32, tag="actscr")
            s3pre = sb.tile([128, NB * CO], BF16, tag="s3pre")
            z3pre = sb.tile([C, 1], F32, tag="z3pre")

            fence_in = dram.tile([8, 1], BF16, name="fence_in", tag="fence_in")
            fence_out = dram.tile([8, 1], BF16, name="fence_out", tag="fence_out")

            # remote-exchange semaphores (SPMD: same numbers on every core)
            sem_r1 = nc.alloc_semaphore("xch1_arrive")
            sem_l1 = nc.alloc_semaphore("xch1_sent")
            sem_c1 = nc.alloc_semaphore("xch1_conf")
            sem_lc1 = nc.alloc_semaphore("xch1_conf_sent")
            sem_r2 = nc.alloc_semaphore("xch2_arrive")
            sem_l2 = nc.alloc_semaphore("xch2_sent")

            # ---- init fence: tiny AllToAll on the CC stream.  Its one-time
            # ncfw init barrier synchronizes all cores past their preamble
            # semaphore-clear, making remote sem increments safe to send.
            # The input is intentionally uninitialized (bypass collective,
            # value unused) so the trigger has no deps and fires first.
            fence_cc = nc.gpsimd.collective_compute(
                "AllToAll", mybir.AluOpType.bypass,
                replica_groups=[list(range(NCORES))],
                ins=[fence_in[:].opt()], outs=[fence_out[:].opt()])
            nc.gpsimd.memset(onesb[:], 1.0)

            # ---- loads: merged big DMAs spread across engine queues ----
            for h in range(3):
                ts_, te_ = 3 * h, 3 * (h + 1)
                nc.sync.dma_start(
                    u1sb[:, ts_ * B:te_ * B].rearrange("p (t b) -> p t b", b=B),
                    u1d[ts_ * 128:te_ * 128, :].rearrange("(t p) b -> p t b", p=128))
                nc.scalar.dma_start(
                    w1sb[:, ts_ * CO:te_ * CO].rearrange("p (t f) -> p t f", f=CO),
                    w1d[ts_ * 128:te_ * 128, :].rearrange("(t p) f -> p t f", p=128))
            nc.scalar.dma_start(idsb[:], identd[:, :])
            nc.sync.dma_start(rsb[:], rmatd[:, :])
            for g in range(2):
                nc.scalar.dma_start(selsb[g][:], seld[g * 80:(g + 1) * 80, :])
            nc.vector.memset(cexp[:], 1.0)
            nc.vector.memset(ones10[:], 1.0)
            nc.vector.memset(spre2[:, F1:F2], 0.0)

            # ---- remote-exchange descriptor prep (fires at trigger time) ----
            # ONE broadcast per exchange with all 8 destinations real: dummy
            # slot descriptors cost as much engine time as real ones (~97ns
            # each), so 7 single-dest broadcasts take ~45us while one all-real
            # broadcast moves the same data in ~6us.  Each sender writes slot
            # <own core id> on every receiver via a register-offset out_ap
            # (the only sender-dependent quantity available in SPMD).
            pid_sv = nc.gpsimd.partition_id()
            offr1 = nc.gpsimd.alloc_register("slot_off1")
            nc.gpsimd.reg_mul(offr1, pid_sv, F1)
            offr2 = nc.gpsimd.alloc_register("slot_off2")
            nc.gpsimd.reg_mul(offr2, pid_sv, F2)

            def prep_exchange(gbuf, spre, ncols, sem_r, sem_l, queue, offr,
                              after=None):
                base = gbuf[:, 0:ncols]
                out_dyn = bass.AP(base.tensor, offr, base.ap,
                                  dep_tracking_offset=0)
                p = nc.gpsimd.remote_dma_broadcast(
                    out_dyn, spre[:, 0:ncols], sem_r, sem_l,
                    rdests=[(0, k) for k in range(8)], queue_num=queue)
                if after is not None:
                    # keep the SWDGE ring strictly [prep1, trig1, prep2,
                    # trig2]: a batch-2 prep scheduled before trig1 would
                    # make trig1 fire the wrong descriptors.
                    bass._add_dep_helper(
                        p.ins, after.ins, sync=False,
                        reason="ring order: batch-2 preps after trig1")

            prep_exchange(gbuf1, spre1, F1, sem_r1, sem_l1, 1, offr1)

            def build_transposes():
                # u2 = u1^T and w2 = w1^T, built on-device via PE transposes
                # during the exchange window.  Halves the host->device upload.
                for t in range(NT):
                    for nb in range(NB):
                        tp = ps.tile([128, 128], BF16, name="tpu", tag="pbig")
                        nc.tensor.transpose(
                            tp[:],
                            u1sb[:, t * B + nb * 128: t * B + (nb + 1) * 128],
                            idsb[:])
                        nc.vector.tensor_copy(
                            u2sb[:, nb * IK + t * 128: nb * IK + (t + 1) * 128],
                            tp[:])
                for t in range(NT):
                    for g in range(2):
                        tp = ps.tile([128, 128], BF16, name="tpw", tag="pbig")
                        nc.tensor.transpose(
                            tp[0:80, :],
                            w1sb[:, t * CO + g * 80: t * CO + (g + 1) * 80],
                            idsb[:])
                        nc.vector.tensor_copy(
                            w2sb[g][:, t * 128:(t + 1) * 128], tp[0:80, :])

            EXP = mybir.ActivationFunctionType.Exp
            SQRT = mybir.ActivationFunctionType.Sqrt

            def s_matmul(wt):
                stiles = [ps.tile([128, CO], F32, name="spsum", tag="pbig") for _ in range(NB)]
                for nb in range(NB):
                    for t in range(NT):
                        nc.tensor.matmul(
                            stiles[nb][:],
                            u1sb[:, t * B + nb * 128: t * B + (nb + 1) * 128],
                            wt[:, t * CO:(t + 1) * CO],
                            start=(t == 0), stop=(t == NT - 1))
                return stiles

            def z_matmul():
                z = ps2.tile([C, 1], F32, name="zpsum", tag="psmall")
                for t in range(NT):
                    nc.tensor.matmul(z[:], cexp[:, t * C:(t + 1) * C], onesb[:],
                                     start=(t == 0), stop=(t == NT - 1))
                return z

            # arrival waits are attached to the guard nops AFTER the
            # TileContext exits: the single-core scheduling simulator cannot
            # model remote semaphore increments and would report a deadlock.
            post_waits = []

            def pack_trigger(stiles, z, spre, gbuf, ncols, queue, gate=None):
                # pack partials into the send tile
                packs = []
                for nb in range(NB):
                    scol = slice(nb * CO, (nb + 1) * CO)
                    packs.append(nc.vector.tensor_copy(spre[:, scol],
                                                       stiles[nb][:]))
                if z is not None:
                    packs.append(nc.vector.tensor_copy(spre[0:C, F1:F1 + 1],
                                                       z[:]))
                # fire the prepped remote sends.  The DMA reads spre at
                # trigger time, but Tile attributed the RAW dep to the prep
                # (which traced before the producers), so the trigger needs
                # explicit sync deps on the pack copies.
                trig = nc.gpsimd.trigger_dma(count=None, queue_num=queue)
                for p in packs:
                    bass._add_dep_helper(
                        trig.ins, p.ins, sync=True,
                        reason="trigger reads spre: wait for pack copies")
                if gate is not None:
                    bass._add_dep_helper(
                        trig.ins, gate.ins, sync=True,
                        reason="remote sends gated on ncfw init fence")
                # prefetch the Sqrt activation table while the exchange runs
                nc.vector.tensor_tensor(actscr[:], spre[0:1, 0:1],
                                        spre[0:1, 0:1], op=MULT)
                nc.scalar.activation(actscr[:], actscr[:], SQRT)
                return trig

            def sum_slots(trig, gbuf, ncols, sem_r, dst):
                # arrival guard: DVE nop that (post-scheduling) waits for all
                # 7 remote transfers; ordered after the trigger so it cannot
                # stall the sends it waits on.  Two chained nops: Tile's
                # cross-engine trigger-order wait lands on the first, leaving
                # the second's wait slot free for the remote-arrival wait.
                order = nc.vector.nop(hint="xch_order", nofuse=True)
                bass._add_dep_helper(
                    order.ins, trig.ins, sync=True,
                    reason="arrival guard ordered after remote trigger")
                guard = nc.vector.nop(hint="xch_arrival", nofuse=True)
                bass._add_dep_helper(
                    guard.ins, order.ins, sync=False,
                    reason="arrival guard after order nop")
                post_waits.append((guard, sem_r, 16))
                # tree-sum the 8 slots; runs after the guard on the DVE queue
                a1 = nc.vector.tensor_tensor(
                    gbuf[:, 0:4 * ncols], gbuf[:, 0:4 * ncols],
                    gbuf[:, 4 * ncols:8 * ncols], op=ADD)
                bass._add_dep_helper(
                    a1.ins, guard.ins, sync=False,
                    reason="slot sum gated on arrival guard")
                nc.vector.tensor_tensor(
                    gbuf[:, 0:2 * ncols], gbuf[:, 0:2 * ncols],
                    gbuf[:, 2 * ncols:4 * ncols], op=ADD)
                nc.vector.tensor_tensor(
                    dst[:, 0:ncols], gbuf[:, 0:ncols],
                    gbuf[:, ncols:2 * ncols], op=ADD)

            def squash(src, const_q=False):
                # W1 is host-prescaled by 1/I, so iter-1 needs no q scale;
                # later iterations use q = I*NCORES/Z_glob (= (1/I)/Ztrue).
                if not const_q:
                    nc.vector.tensor_scalar_mul(qsb[:], qsb[:],
                                                float(NCORES) * float(I))
                last_sqrt = None
                for nb in range(NB):
                    scol = slice(nb * CO, (nb + 1) * CO)
                    ocol = slice(nb * O, (nb + 1) * O)
                    s4 = src[:, scol].rearrange("p (c o) -> p c o", o=O)
                    if not const_q:
                        q4 = qsb[:].unsqueeze(2).broadcast_to([128, C, O])
                        nc.vector.tensor_tensor(s4, s4, q4, op=MULT)
                    nc.vector.tensor_tensor(sq[:, scol], src[:, scol],
                                            src[:, scol], op=MULT)
                    nc.vector.tensor_reduce(
                        n2[:, ocol],
                        sq[:, scol].rearrange("p (c o) -> p o c", o=O),
                        axis=mybir.AxisListType.X, op=ADD)
                    # phi = sqrt(n2)/(1+n2)  (1e-10 guard dropped)
                    last_sqrt = nc.scalar.activation(phi[:, ocol], n2[:, ocol],
                                                     SQRT)
                    nc.vector.tensor_scalar_add(n2[:, ocol], n2[:, ocol], 1.0)
                    nc.vector.reciprocal(n2[:, ocol], n2[:, ocol])
                    nc.vector.tensor_tensor(phi[:, ocol], phi[:, ocol],
                                            n2[:, ocol], op=MULT)
                    p4 = phi[:, ocol].unsqueeze(1).broadcast_to([128, C, O])
                    o4 = outc[:, scol].rearrange("p (c o) -> p c o", o=O)
                    nc.vector.tensor_tensor(o4, s4, p4, op=MULT)
                # prefetch the Exp table (used by agr tail) while PE runs
                # T; ordered after the last sqrt so the 1.5us table load can
                # never wedge between the two squash chunks' activations.
                pre = nc.scalar.activation(actscr[:], phi[0:1, 0:1], EXP)
                bass._add_dep_helper(
                    pre.ins, last_sqrt.ins, sync=False,
                    reason="exp table prefetch after last squash sqrt")

            def agr_phase(cwsrc):
                # T[co,ik] = sum_b out[b,co] u2[b,ik]; M = W2 .* T
                Tgs = [ps.tile([80, IK], F32, name=f"Tpsum{g}", tag="pbig")
                       for g in range(2)]
                if SIM_SAFE:
                    for g in range(2):
                        for c0, cn in ((0, 512), (512, 512), (1024, 128)):
                            for nb in range(NB):
                                nc.tensor.matmul(
                                    Tgs[g][:, c0:c0 + cn],
                                    outc[:, nb * CO + g * 80:
                                         nb * CO + (g + 1) * 80],
                                    u2sb[:, nb * IK + c0: nb * IK + c0 + cn],
                                    start=(nb == 0), stop=(nb == NB - 1))
                else:
                    for nb in range(NB):
                        for g in range(2):
                            for c0, cn in ((0, 512), (512, 512), (1024, 128)):
                                nc.tensor.matmul(
                                    Tgs[g][:, c0:c0 + cn],
                                    outc[:, nb * CO + g * 80:
                                         nb * CO + (g + 1) * 80],
                                    u2sb[:, nb * IK + c0: nb * IK + c0 + cn],
                                    start=(nb == 0), stop=(nb == NB - 1),
                                    skip_group_check=True)
                for c0, cn in ((0, 512), (512, 512), (1024, 128)):
                    for g in range(2):
                        nc.vector.tensor_tensor(msb[g][:, c0:c0 + cn],
                                                w2sb[g][:, c0:c0 + cn],
                                                Tgs[g][:, c0:c0 + cn], op=MULT)
                # all 9 tiles' agr_k into ONE psum tile (disjoint col ranges),
                # then single wide ops for copy / k-sum / exp / cW update
                agrk = ps2.tile([128, NT * C], F32, name="agrkp", tag="psmall")
                for t in range(NT):
                    tcol = slice(t * C, (t + 1) * C)
                    for g in range(2):
                        nc.tensor.matmul(agrk[:, tcol],
                                         msb[g][:, t * 128:(t + 1) * 128],
                                         selsb[g][:], start=(g == 0), stop=(g == 1))
                nc.vector.tensor_copy(agrk_sb[:], agrk[:])
                repl = ps2.tile([128, NT * C], F32, name="replp", tag="psmall")
                nc.tensor.matmul(repl[:], rsb[:], agrk_sb[:])
                # multiplicative routing state: exp(agr) straight from PSUM,
                # then cW *= efac (per 3-tile chunk so s-matmuls start early)
                nc.scalar.activation(efac[:], repl[:], EXP)
                for h in range(3):
                    ts_, te_ = 3 * h, 3 * (h + 1)
                    cw4 = cwsb[:, ts_ * CO:te_ * CO].rearrange(
                        "p (t c o) -> p t c o", c=C, o=O)
                    w14 = cwsrc[:, ts_ * CO:te_ * CO].rearrange(
                        "p (t c o) -> p t c o", c=C, o=O)
                    ce4 = efac[:, ts_ * C:te_ * C].rearrange(
                        "p (t c) -> p t c", c=C) \
                        .unsqueeze(3).broadcast_to([128, 3, C, O])
                    nc.vector.tensor_tensor(cw4, w14, ce4, op=MULT)
                # cexp (softmax numerator, for Z) updates multiplicatively too
                nc.vector.tensor_tensor(cexp[:], cexp[:], efac[:], op=MULT)

            # ================= iteration 1 =================
            with nc.named_scope("it1_s"):
                stiles = s_matmul(w1sb)       # cexp == 1 -> cW == W1
            with nc.named_scope("xch1"):
                trig1 = pack_trigger(stiles, None, spre1, gbuf1, F1, 1)
                # The data trigger MUST be gated on the fence: ungated early
                # sends land while a skewed peer may still be running another
                # NEFF (e.g. the caller's earlier jax ops) whose SBUF usage
                # clobbers the landed data -> silent wrong sums (observed
                # rel err 0.23 once in ~12 runs with ungated sends).
                bass._add_dep_helper(
                    trig1.ins, fence_cc.ins, sync=True,
                    reason="remote data sends gated on ncfw init fence")
            # transposes run during the fence/exchange window; their DVE
            # psum->sbuf copies are queued before the arrival guard so they
            # cannot stall behind it.
            with nc.named_scope("transp"):
                build_transposes()
            with nc.named_scope("xch1_sum"):
                sum_slots(trig1, gbuf1, F1, sem_r1, ssb)
            with nc.named_scope("it1_squash"):
                squash(ssb, const_q=True)
            with nc.named_scope("it1_agr"):
                agr_phase(w1sb)
            # ================= iteration 2 =================
            # xch2 preps go on the same queue, emitted after trig1 so
            # trigger_dma(count=None) picks up exactly this batch.
            prep_exchange(gbuf2, spre2, F2, sem_r2, sem_l2, 1, offr2,
                          after=trig1)
            with nc.named_scope("it2_s"):
                stiles = s_matmul(cwsb)
            with nc.named_scope("it2_z"):
                z = z_matmul()
            with nc.named_scope("xch2"):
                trig2 = pack_trigger(stiles, z, spre2, gbuf2, F2, 1)
                sum_slots(trig2, gbuf2, F2, sem_r2, ssb2)
                # q = NCORES*I / Z: build diag(z), PE-broadcast to 128
                # partitions, then DVE reciprocal (replaces the slow DRE
                # partition-broadcast DMA of the collective version).
                nc.vector.tensor_tensor(
                    zdsb[0:C, 0:C], idsb[0:C, 0:C],
                    ssb2[0:C, F1:F1 + 1].broadcast_to([C, C]), op=MULT)
                qp = ps2.tile([128, C], F32, name="qpsum", tag="psmall")
                nc.tensor.matmul(qp[:], ones10[0:C, :], zdsb[0:C, 0:C])
                nc.vector.reciprocal(qsb[:], qp[:])
            with nc.named_scope("it2_squash"):
                squash(ssb2)
            with nc.named_scope("it2_agr"):
                agr_phase(cwsb)
            # ================= iteration 3 (s~ partial only) =================
            stiles = s_matmul(cwsb)
            z = z_matmul()
            for nb in range(NB):
                scol = slice(nb * CO, (nb + 1) * CO)
                nc.vector.tensor_copy(s3pre[:, scol], stiles[nb][:])
                nc.sync.dma_start(s3d[nb * 128:(nb + 1) * 128, :],
                                  s3pre[:, scol])
            nc.vector.tensor_copy(z3pre[:], z[:])
            nc.sync.dma_start(z3d[:, :], z3pre[:])
            if DEBUG_GBUF:
                nc.scalar.dma_start(sdbg[:, :], ssb[:, :])
                nc.scalar.dma_start(s2dbg[:, :], ssb2[:, :])
                nc.scalar.dma_start(odbg[:, :], outc[:, :])
                nc.scalar.dma_start(qdbg[:, :], qsb[:, :])
                # dump raw gather buffer BEFORE the tree-sum mangles it: the
                # sum overwrote slots 0-3, so instead re-copy slots 4-7 plus
                # snapshot: simplest is dumping the whole (mangled) buffer;
                # slots 4..7 are untouched by the tree-sum.
                nc.sync.dma_start(gdbg[:, :], gbuf1[:, :])
                nc.sync.dma_start(gdbg2[:, :], gbuf2[:, :])

    # Attach the remote-arrival waits now that Tile's scheduling sim has run
    # (it cannot model increments arriving from peer cores).  The guards are
    # already ordered trigger -> guard -> slot-sum by explicit dep edges, so
    # the waits only ever delay instructions that need the remote data.
    for guard, sem, val in post_waits:
        guard.wait_op(sem, val, "sem-ge")

    nc.compile()
    return nc


def _get_nc():
    if "nc" not in _CACHE:
        _CACHE["nc"] = _build()
    return _CACHE["nc"]


def _host_inputs(x, weights):
    x = np.ascontiguousarray(x, dtype=np.float32)
    weights = np.ascontiguousarray(weights, dtype=np.float32)
    rmat = np.kron(np.eye(16, dtype=np.float32), np.ones((8, 8), np.float32))
    # w1 is prescaled by 1/I and w2 is its on-device transpose, so sel
    # carries I/B (instead of 1/B) to compensate in the agreement path.
    sel = np.zeros((CO, C), np.float32)
    for c in range(C):
        sel[c * O:(c + 1) * O, c] = np.float32(I) / np.float32(B)
    ident = np.eye(128, dtype=np.float32)
    in_maps = []
    for m in range(NCORES):
        sl = slice(m * ILOC, (m + 1) * ILOC)
        xs = x[:, :, sl]                          # [B, K, ILOC]
        ws = weights[sl]                          # [ILOC, C, O, K]
        in_maps.append({
            "u1": np.ascontiguousarray(xs.transpose(2, 1, 0).reshape(IK, B)).astype(BF),
            "w1": (np.ascontiguousarray(
                ws.transpose(0, 3, 1, 2).reshape(IK, CO)) / np.float32(I)).astype(BF),
            "rmat": rmat.astype(BF),
            "sel": sel.astype(BF),
            "ident": ident.astype(BF),
        })
    return in_maps


def kernel(x, weights):
    nc = _get_nc()
    in_maps = _host_inputs(x, weights)
    trace = bool(int(os.environ.get("KERNEL_TRACE", "0")))
    res = bass_utils.run_bass_kernel_spmd(
        nc, in_maps, core_ids=list(range(NCORES)), trace=trace)
    if trace and res.exec_time_ns is not None:
        _CACHE["exec_time_ns"] = res.exec_time_ns
        _CACHE["results"] = res
    s3 = np.zeros((B, CO), np.float64)
    z8 = np.zeros((C,), np.float64)
    for r in res.results:
        s3 += np.asarray(r["s3out"], dtype=np.float64)
        z8 += np.asarray(r["z3out"], dtype=np.float64)[:, 0]
    s = (s3.reshape(B, C, O) * (float(I) * NCORES / z8)[None, :, None]).astype(np.float32)
    nsq = (s * s).sum(1, keepdims=True)
    out = s * (nsq / (1.0 + nsq)) / (np.sqrt(nsq) + 1e-10)
    return out[..., None].astype(np.float32)
